# revision 1
# baseline (speedup 1.0000x reference)
"""Additive-attention pooling kernel for TRN2, data-parallel over batch on 8 cores.

Reference computation (per batch b):
    att[s, o]  = tanh(sum_h x[s,b,h] * W[o,h] + bias[o])
    sim[s]     = sum_o att[s, o] * context[o]
    e[s]       = exp(sim[s])            (softmax without max-subtraction;
                                         |sim| <= sum|ctx| <= 22.6 so exp is safe)
    out[b, h]  = sum_s x[s,b,h] * e[s] / sum_s e[s]

Per-core dataflow (4 batches per core):
  - x shard is host-transposed to xT[h, b, s] so the GEMM contraction (h) is on
    partitions.  GEMM: psum[o_chunk, s_tile] = WT_chunk.T @ xT_chunk (4 K-chunks).
  - ACT applies tanh with per-partition bias while evacuating PSUM -> SBUF.
  - sim: PE matmul with context chunk as stationary [128,1] -> psum[1, s_tile].
  - ACT Exp with accum_out accumulates the denominator per s-tile.
  - e is broadcast to 128 partitions with a K=1 outer-product matmul.
  - pooling: DVE tensor_tensor_reduce(xT_tile * e_bcast) accum -> numerator.
  - epilogue per batch: reduce denominator, reciprocal, broadcast via K=1
    matmul, scale numerator, DMA out.
"""

import sys

if "/opt/trn_rl_repo" not in sys.path:
    sys.path.insert(0, "/opt/trn_rl_repo")

import numpy as np

SEQ, BATCH, HID = 4096, 32, 512
NCORES = 8
BPC = BATCH // NCORES          # batches per core
ST = 512                       # s-tile width
NST = SEQ // ST                # 8 s-tiles per batch
NHC = HID // 128               # 4 h chunks
NOC = HID // 128               # 4 o chunks

# GEMM input mode: "f32r" (full-rate fp32 matmul, inputs declared float32r),
# "f32" (exact, 1/4-rate), "bf16" (host-cast, half DMA traffic)
GEMM_MODE = "bf16"
# "ttr" = fused tensor_tensor_reduce pooling; "safe" = tensor_tensor + reduce
POOL_MODE = "safe"
# V2: bf16 act/ctx/e/ones so sim + ebc matmuls run at 1 cyc/row, and the
# pooling multiply/reduce runs in DVE 2x mode from an SBUF bf16 broadcast
OPT_V2 = True

_CACHE = {}


def _build(mode, pool_mode=None):
    if pool_mode is None:
        pool_mode = POOL_MODE
    import concourse.tile as tile
    from concourse import bacc, mybir

    F32 = mybir.dt.float32
    F32R = mybir.dt.float32r
    BF16 = mybir.dt.bfloat16
    AF = mybir.ActivationFunctionType
    ALU = mybir.AluOpType

    xdt = {"f32r": F32R, "f32": F32, "bf16": BF16}[mode]

    nc = bacc.Bacc(
        "TRN2",
        target_bir_lowering=False,
        debug=False,
        enable_asserts=True,
        num_devices=NCORES,
    )

    ADT = BF16 if OPT_V2 else F32  # dtype of the act/ctx/e/ones path

    xt_d = nc.dram_tensor("xt", (HID, BPC, SEQ), xdt, kind="ExternalInput").ap()
    wt_d = nc.dram_tensor("wt", (128, NHC * HID), xdt, kind="ExternalInput").ap()
    b_d = nc.dram_tensor("bvec", (128, NOC), F32, kind="ExternalInput").ap()
    ctx_d = nc.dram_tensor("ctx", (128, NOC), ADT, kind="ExternalInput").ap()
    # host-supplied constants: [:, :128] all-ones, [:, 128:] zeros
    const_d = nc.dram_tensor(
        "consts", (128, 128 + ST + 1), ADT, kind="ExternalInput"
    ).ap()
    out_d = nc.dram_tensor("out", (BPC, HID), F32, kind="ExternalOutput").ap()

    with tile.TileContext(nc) as tc:
        with (
            tc.tile_pool(name="consts", bufs=1) as cpool,
            tc.tile_pool(name="xs", bufs=2) as xpool,
            tc.tile_pool(name="acts", bufs=8) as apool,
            tc.tile_pool(name="es", bufs=4) as epool,
            tc.tile_pool(name="junks", bufs=4) as jpool,
            tc.tile_pool(name="small", bufs=2) as spool,
            tc.tile_pool(name="attps", bufs=4, space="PSUM") as ps_att,
            tc.tile_pool(name="simps", bufs=2, space="PSUM") as ps_sim,
            tc.tile_pool(name="ebcps", bufs=2, space="PSUM") as ps_ebc,
        ):
            wt_sb = cpool.tile([128, NHC * HID], xdt, tag="wt")
            b_sb = cpool.tile([128, NOC], F32, tag="b")
            ctx_sb = cpool.tile([128, NOC], ADT, tag="ctx")
            # all-ones stationary for K=128 partition-broadcast matmuls
            ones_mat = cpool.tile([128, 128], ADT, tag="ones")
            # e lives on partition 0 of a zeroed [128, ST] tile so that
            # ones_mat.T @ e_full column-sums to a partition-broadcast of e
            e_full = cpool.tile([128, ST], ADT, tag="efull")
            nc.sync.dma_start(wt_sb[:], wt_d)
            nc.sync.dma_start(b_sb[:], b_d)
            nc.sync.dma_start(ctx_sb[:], ctx_d)
            nc.sync.dma_start(ones_mat[:], const_d[:, 0:128])
            nc.sync.dma_start(e_full[:], const_d[:, 128 : 128 + ST])

            for b in range(BPC):
                xs = []
                for hc in range(NHC):
                    xtile = xpool.tile([128, SEQ], xdt, tag=f"x{hc}")
                    nc.sync.dma_start(
                        xtile[:], xt_d[hc * 128 : (hc + 1) * 128, b, :]
                    )
                    xs.append(xtile)

                num_cols = spool.tile([128, NHC * NST], F32, tag="num")
                den_cols = spool.tile([128, NST], F32, tag="den")

                for st in range(NST):
                    ssl = slice(st * ST, (st + 1) * ST)
                    acts = []
                    for oc in range(NOC):
                        attps = ps_att.tile([128, ST], F32, tag="att")
                        for hc in range(NHC):
                            nc.tensor.matmul(
                                attps[:],
                                wt_sb[:, hc * HID + oc * 128 : hc * HID + (oc + 1) * 128],
                                xs[hc][:, ssl],
                                start=(hc == 0),
                                stop=(hc == NHC - 1),
                            )
                        act = apool.tile([128, ST], ADT, tag="act")
                        nc.scalar.activation(
                            act[:], attps[:], AF.Tanh, bias=b_sb[:, oc : oc + 1]
                        )
                        acts.append(act)

                    simps = ps_sim.tile([1, ST], F32, tag="sim")
                    for oc in range(NOC):
                        nc.tensor.matmul(
                            simps[:],
                            ctx_sb[:, oc : oc + 1],
                            acts[oc][:],
                            start=(oc == 0),
                            stop=(oc == NOC - 1),
                        )

                    nc.scalar.activation(e_full[0:1, :], simps[:], AF.Exp)

                    ebcps = ps_ebc.tile([128, ST], F32, tag="ebc")
                    nc.tensor.matmul(
                        ebcps[:], ones_mat[:], e_full[:], start=True, stop=True
                    )
                    # evict broadcast e to SBUF (bf16) so pooling runs in DVE
                    # 2x mode; every partition holds the same e row
                    ebc_sb = epool.tile([128, ST], ADT, tag="ebcsb")
                    nc.scalar.activation(ebc_sb[:], ebcps[:], AF.Copy)
                    # denominator partial sums land replicated per-partition
                    nc.vector.reduce_sum(
                        out=den_cols[:, st : st + 1], in_=ebc_sb[:],
                        axis=mybir.AxisListType.X,
                    )

                    for hc in range(NHC):
                        junk = jpool.tile([128, ST], ADT, tag="junk")
                        xin = (
                            xs[hc][:, ssl].bitcast(F32)
                            if mode == "f32r"
                            else xs[hc][:, ssl]
                        )
                        ncol = num_cols[:, hc * NST + st : hc * NST + st + 1]
                        if pool_mode == "ttr":
                            nc.vector.tensor_tensor_reduce(
                                out=junk[:],
                                in0=xin,
                                in1=ebc_sb[:],
                                scale=1.0,
                                scalar=0.0,
                                op0=ALU.mult,
                                op1=ALU.add,
                                accum_out=ncol,
                            )
                        else:
                            nc.vector.tensor_mul(junk[:], xin, ebc_sb[:])
                            nc.vector.reduce_sum(
                                out=ncol, in_=junk[:], axis=mybir.AxisListType.X
                            )

                # batch epilogue: denominator is already per-partition
                den_col = spool.tile([128, 1], F32, tag="den1")
                nc.vector.reduce_sum(
                    out=den_col[:], in_=den_cols[:], axis=mybir.AxisListType.X
                )
                recip_sb = spool.tile([128, 1], F32, tag="recip")
                nc.vector.reciprocal(recip_sb[:], den_col[:])

                att_red = spool.tile([128, NHC], F32, tag="attred")
                for hc in range(NHC):
                    nc.vector.reduce_sum(
                        out=att_red[:, hc : hc + 1],
                        in_=num_cols[:, hc * NST : (hc + 1) * NST],
                        axis=mybir.AxisListType.X,
                    )
                att_fin = spool.tile([128, NHC], F32, tag="attfin")
                nc.scalar.activation(
                    att_fin[:], att_red[:], AF.Copy, scale=recip_sb[:]
                )

                nc.sync.dma_start(
                    out_d.rearrange("n (c p) -> n p c", p=128)[b], att_fin[:]
                )

    nc.compile()
    return nc


def _get_nc(mode):
    key = (mode, POOL_MODE)
    if key not in _CACHE:
        _CACHE[key] = _build(mode, POOL_MODE)
    return _CACHE[key]


def _prepare_in_maps(x, W, b, context, mode):
    x = np.asarray(x, dtype=np.float32)
    W = np.asarray(W, dtype=np.float32)
    b = np.asarray(b, dtype=np.float32)
    context = np.asarray(context, dtype=np.float32)

    # wt[p, hc*HID + o] = W[o, hc*128 + p]
    wt_host = np.ascontiguousarray(
        W.T.reshape(NHC, 128, HID).transpose(1, 0, 2).reshape(128, NHC * HID)
    )
    b_host = np.ascontiguousarray(b.reshape(NOC, 128).T)
    ctx_host = np.ascontiguousarray(context.reshape(NOC, 128).T)
    const_host = np.zeros((128, 128 + ST + 1), dtype=np.float32)
    const_host[:, 0:128] = 1.0
    if OPT_V2:
        import ml_dtypes

        ctx_host = ctx_host.astype(ml_dtypes.bfloat16)
        const_host = const_host.astype(ml_dtypes.bfloat16)

    if mode == "bf16":
        import ml_dtypes

        wt_host = wt_host.astype(ml_dtypes.bfloat16)

    in_maps = []
    for c in range(NCORES):
        xs = x[:, c * BPC : (c + 1) * BPC, :]          # [SEQ, BPC, HID]
        xT = np.ascontiguousarray(xs.transpose(2, 1, 0))  # [HID, BPC, SEQ]
        if mode == "bf16":
            import ml_dtypes

            xT = xT.astype(ml_dtypes.bfloat16)
        in_maps.append(
            {"xt": xT, "wt": wt_host, "bvec": b_host, "ctx": ctx_host,
             "consts": const_host}
        )
    return in_maps


def kernel(x, W, b, context, _trace=False):
    from concourse.bass_utils import run_bass_kernel_spmd

    mode = GEMM_MODE
    nc = _get_nc(mode)
    in_maps = _prepare_in_maps(x, W, b, context, mode)

    res = run_bass_kernel_spmd(
        nc, in_maps, core_ids=list(range(NCORES)), trace=_trace
    )
    out = np.concatenate(
        [res.results[c]["out"] for c in range(NCORES)], axis=0
    )
    out = out.astype(np.float32)
    if _trace:
        return out, res
    return out


if __name__ == "__main__":
    rng = np.random.default_rng(0)
    x = rng.standard_normal((SEQ, BATCH, HID), dtype=np.float32)
    W = rng.standard_normal((HID, HID), dtype=np.float32) / np.sqrt(HID)
    b = rng.standard_normal(HID).astype(np.float32) * 0.04
    c = rng.standard_normal(HID).astype(np.float32) * 0.04
    out = kernel(x=x, W=W, b=b, context=c)
    print(out.shape, out.dtype)

